# revision 4
# baseline (speedup 1.0000x reference)
"""Multi-head attention (B=2, S=2048, D=1024, H=16) on 8 Trainium2 cores.

Sharding: core c handles batch b=c//4, head-group g=c%4 (4 heads = 256 QKV
columns).  Each core computes its head-group's QKV projections, attention,
and a partial output projection (its 256 rows of Wo); the host sums the 4
partials per batch and adds bo.

Device layouts (per core):
  xqT/xkT/xvT [1024, 2048]  inputs pre-transposed on host (d_model on rows)
  Qt, Kt      [dcol, s]     SBUF [128, 2, 2048] (head-pair h%2 stacked in
                            partition halves: head h lives at partitions
                            (h%2)*64..+64, free-tile h//2)
  V           [s, dcol]     SBUF f16 [128, 16, 4, 65]; col 64 = 1.0 (ones
                            column makes att@V also produce the softmax
                            denominator)
  S^T = Kt'Q  PSUM [k, q]   exp on ScalarE (scale=1/8) -> P^T f16
  O = P'V     PSUM [q, 65]  normalize by col 64 on VectorE -> A [s, 256]
  A^T via PE transpose, out = A^T' Wo -> partial [2048, 1024]
"""

import sys

sys.path.insert(0, "/opt/trn_rl_repo")

import numpy as np

import concourse.bacc as bacc
import concourse.mybir as mybir
from concourse import bass_utils, tile
from concourse.masks import make_identity

P = 128
S = 2048  # sequence length
DM = 1024  # d_model
GC = 256  # QKV columns per core (4 heads x 64)
NHC = 4  # heads per core
DEP = 64  # head depth
NKT = DM // P  # 8 contraction tiles over d_model
NST = S // P  # 16 s-tiles of 128
NSC = S // 512  # 4 s-chunks of 512
f32 = mybir.dt.float32
f32r = mybir.dt.float32r
f16 = mybir.dt.float16
EXP = mybir.ActivationFunctionType.Exp

_nc_cache = None


def r(ap):
    return ap.bitcast(f32r)


def build_nc():
    global _nc_cache
    if _nc_cache is not None:
        return _nc_cache
    from contextlib import ExitStack

    nc = bacc.Bacc("TRN2", target_bir_lowering=False, debug=False)
    xqT = nc.dram_tensor("xqT", [DM, S], f32r, kind="ExternalInput").ap()
    xkT = nc.dram_tensor("xkT", [DM, S], f32r, kind="ExternalInput").ap()
    xvT = nc.dram_tensor("xvT", [DM, S], f32r, kind="ExternalInput").ap()
    wq = nc.dram_tensor("wq", [DM, GC], f32r, kind="ExternalInput").ap()
    wk = nc.dram_tensor("wk", [DM, GC], f32r, kind="ExternalInput").ap()
    wv = nc.dram_tensor("wv", [DM, GC], f32r, kind="ExternalInput").ap()
    wo = nc.dram_tensor("wo", [GC, DM], f32r, kind="ExternalInput").ap()
    bq = nc.dram_tensor("bq", [GC], f32, kind="ExternalInput").ap()
    bk = nc.dram_tensor("bk", [GC], f32, kind="ExternalInput").ap()
    bv = nc.dram_tensor("bv", [GC], f32, kind="ExternalInput").ap()
    out = nc.dram_tensor("out", [S, DM], f32, kind="ExternalOutput").ap()

    with tile.TileContext(nc) as tc, ExitStack() as ctx:
        consts = ctx.enter_context(tc.tile_pool(name="consts", bufs=1))
        wq_sb = consts.tile([P, NKT, GC], f32r, tag="wq")
        wk_sb = consts.tile([P, NKT, GC], f32r, tag="wk")
        wv_sb = consts.tile([P, NKT, GC], f32r, tag="wv")
        for w_sb, w in ((wq_sb, wq), (wk_sb, wk), (wv_sb, wv)):
            nc.sync.dma_start(w_sb[:], w.rearrange("(kt p) m -> p kt m", p=P))
        wo_sb = consts.tile([P, 2, DM], f32r, tag="wo")
        nc.sync.dma_start(wo_sb[:], wo.rearrange("(mt p) n -> p mt n", p=P))

        # Biases as K=128 matmul operands: bias data in partition-row 0,
        # zeros elsewhere, so accumulation stays in 128x128 array mode.
        bq_sb = consts.tile([P, GC], f32r, tag="bq")
        bk_sb = consts.tile([P, GC], f32r, tag="bk")
        bv_sb = consts.tile([P, GC], f32r, tag="bv")
        stage = consts.tile([P, GC], f32, tag="stage")
        for b_sb, b in ((bq_sb, bq), (bk_sb, bk), (bv_sb, bv)):
            nc.vector.memset(stage[:], 0.0)
            nc.sync.dma_start(stage[0:1, :], b[None, :])
            nc.vector.tensor_copy(b_sb[:], stage[:])
        ones_big = consts.tile([P, 512], f32r, tag="ones")
        nc.vector.memset(stage[:], 1.0)
        nc.vector.tensor_copy(ones_big[:, 0:GC], stage[:])
        nc.vector.tensor_copy(ones_big[:, GC:512], stage[:])
        e0row = consts.tile([P, P], f32r, tag="e0")
        nc.vector.memset(stage[:], 0.0)
        nc.vector.memset(stage[0:1, 0:P], 1.0)
        nc.vector.tensor_copy(e0row[:], stage[:, 0:P])
        ident = consts.tile([P, P], f32, tag="ident")
        make_identity(nc, ident)

        res = ctx.enter_context(tc.tile_pool(name="res", bufs=1))
        qt_sb = res.tile([P, 2, S], f32r, tag="qt")
        kt_sb = res.tile([P, 2, S], f32r, tag="kt")
        v_sb = res.tile([P, NST, NHC, DEP + 1], f16, tag="v")
        a_sb = res.tile([P, NST, GC], f32, tag="a")
        at_sb = res.tile([P, 2, S], f32r, tag="at")
        nc.vector.memset(v_sb[:, :, :, DEP], 1.0)

        xs_pool = ctx.enter_context(tc.tile_pool(name="xs", bufs=4))

        # ---- QKV projections ----
        with tc.tile_pool(name="pj_psum", bufs=4, space="PSUM") as pj_psum:
            # Qt, Kt: [dcol, s] = W' @ X'
            for w_sb, b_sb, src, dst in (
                (wq_sb, bq_sb, xqT, qt_sb),
                (wk_sb, bk_sb, xkT, kt_sb),
            ):
                for ns in range(NSC):
                    ps = [pj_psum.tile([P, 512], f32, tag="pj", name=f"pjq{ns}_{i}") for i in range(2)]
                    for kt in range(NKT):
                        xt = xs_pool.tile([P, 512], f32r, tag="x")
                        nc.sync.dma_start(
                            xt[:], src[kt * P : (kt + 1) * P, ns * 512 : (ns + 1) * 512]
                        )
                        for mt in range(2):
                            nc.tensor.matmul(
                                ps[mt][:],
                                lhsT=(w_sb[:, kt, mt * P : (mt + 1) * P]),
                                rhs=(xt[:]),
                                start=(kt == 0),
                                stop=False,
                            )
                    for mt in range(2):
                        nc.tensor.matmul(
                            ps[mt][:],
                            lhsT=(b_sb[:, mt * P : (mt + 1) * P]),
                            rhs=(ones_big[:]),
                            start=False,
                            stop=True,
                        )
                        nc.vector.tensor_copy(
                            dst[:, mt, ns * 512 : (ns + 1) * 512], ps[mt][:]
                        )
            # V: [s, dcol]
            for sg in range(4):
                psv = [pj_psum.tile([P, GC], f32, tag="pj", name=f"pjv{sg}_{i}") for i in range(4)]
                for kt in range(NKT):
                    xt = xs_pool.tile([P, 512], f32r, tag="x")
                    nc.sync.dma_start(
                        xt[:], xvT[kt * P : (kt + 1) * P, sg * 512 : (sg + 1) * 512]
                    )
                    for j in range(4):
                        nc.tensor.matmul(
                            psv[j][:],
                            lhsT=(xt[:, j * P : (j + 1) * P]),
                            rhs=(wv_sb[:, kt, :]),
                            start=(kt == 0),
                            stop=False,
                        )
                for j in range(4):
                    nc.tensor.matmul(
                        psv[j][:],
                        lhsT=(e0row[:]),
                        rhs=(bv_sb[:]),
                        start=False,
                        stop=True,
                    )
                    nc.vector.tensor_copy(
                        v_sb[:, sg * 4 + j, :, 0:DEP],
                        psv[j][:].rearrange("p (h d) -> p h d", h=NHC),
                    )

        # ---- attention ----
        pt_pool = ctx.enter_context(tc.tile_pool(name="pt", bufs=2))
        rc_pool = ctx.enter_context(tc.tile_pool(name="rc", bufs=8))
        with (
            tc.tile_pool(name="st_psum", bufs=3, space="PSUM") as st_psum,
            tc.tile_pool(name="o_psum", bufs=2, space="PSUM") as o_psum,
        ):
            for h in range(NHC):
                pb = (h % 2) * DEP  # partition base of this head's d-rows
                mt = h // 2
                for qc in range(NSC):
                    ptt = pt_pool.tile([P, NST, 512], f16, tag="pt")
                    for kp in range(NST // 2):
                        stt = st_psum.tile([P, 2, 512], f32, tag="st")
                        for u in range(2):
                            kt = 2 * kp + u
                            nc.tensor.matmul(
                                stt[:, u, :],
                                lhsT=(kt_sb[pb : pb + DEP, mt, kt * P : (kt + 1) * P]),
                                rhs=(qt_sb[pb : pb + DEP, mt, qc * 512 : (qc + 1) * 512]),
                                start=True,
                                stop=True,
                            )
                        nc.scalar.activation(
                            ptt[:, 2 * kp : 2 * kp + 2, :], stt[:], EXP, scale=0.125
                        )
                    ot = o_psum.tile([P, 4, DEP + 1], f32, tag="o")
                    for qt in range(4):
                        for kt in range(NST):
                            nc.tensor.matmul(
                                ot[:, qt, :],
                                lhsT=ptt[:, kt, qt * P : (qt + 1) * P],
                                rhs=v_sb[:, kt, h, :],
                                start=(kt == 0),
                                stop=(kt == NST - 1),
                            )
                    for qt in range(4):
                        st_i = qc * 4 + qt
                        rc = rc_pool.tile([P, 1], f32, tag="rc")
                        nc.vector.reciprocal(rc[:], ot[:, qt, DEP : DEP + 1])
                        nc.vector.tensor_scalar_mul(
                            a_sb[:, st_i, h * DEP : (h + 1) * DEP],
                            ot[:, qt, 0:DEP],
                            rc[:],
                        )

        # ---- transpose A + output projection ----
        ob_pool = ctx.enter_context(tc.tile_pool(name="ob", bufs=3))
        with (
            tc.tile_pool(name="tp_psum", bufs=2, space="PSUM") as tp_psum,
            tc.tile_pool(name="op_psum", bufs=4, space="PSUM") as op_psum,
        ):
            for st in range(NST):
                for mt in range(2):
                    tp = tp_psum.tile([P, P], f32, tag="tp")
                    nc.tensor.transpose(tp[:], a_sb[:, st, mt * P : (mt + 1) * P], ident[:])
                    nc.vector.tensor_copy(at_sb[:, mt, st * P : (st + 1) * P], tp[:])
            for st in range(NST):
                ob = ob_pool.tile([P, DM], f32, tag="ob")
                for oh in range(2):
                    op = op_psum.tile([P, 512], f32, tag="op")
                    for mt in range(2):
                        nc.tensor.matmul(
                            op[:],
                            lhsT=(at_sb[:, mt, st * P : (st + 1) * P]),
                            rhs=(wo_sb[:, mt, oh * 512 : (oh + 1) * 512]),
                            start=(mt == 0),
                            stop=(mt == 1),
                        )
                    nc.scalar.copy(ob[:, oh * 512 : (oh + 1) * 512], op[:])
                nc.sync.dma_start(out[st * P : (st + 1) * P, :], ob[:])

    nc.compile()
    _nc_cache = nc
    return nc


def make_in_maps(query, key, value, Wq, bq, Wk, bk, Wv, bv, Wo, bo):
    qT = [np.ascontiguousarray(query[b].T) for b in range(2)]
    kT = [np.ascontiguousarray(key[b].T) for b in range(2)]
    vT = [np.ascontiguousarray(value[b].T) for b in range(2)]
    in_maps = []
    for c in range(8):
        b, g = divmod(c, 4)
        sl = slice(g * GC, (g + 1) * GC)
        in_maps.append(
            {
                "xqT": qT[b],
                "xkT": kT[b],
                "xvT": vT[b],
                "wq": np.ascontiguousarray(Wq[:, sl]),
                "wk": np.ascontiguousarray(Wk[:, sl]),
                "wv": np.ascontiguousarray(Wv[:, sl]),
                "wo": np.ascontiguousarray(Wo[sl, :]),
                "bq": np.ascontiguousarray(bq[sl]),
                "bk": np.ascontiguousarray(bk[sl]),
                "bv": np.ascontiguousarray(bv[sl]),
            }
        )
    return in_maps


def kernel(query, key, value, Wq, bq, Wk, bk, Wv, bv, Wo, bo):
    query = np.asarray(query, np.float32)
    key = np.asarray(key, np.float32)
    value = np.asarray(value, np.float32)
    nc = build_nc()
    in_maps = make_in_maps(
        np.asarray(query, np.float32),
        np.asarray(key, np.float32),
        np.asarray(value, np.float32),
        np.asarray(Wq, np.float32),
        np.asarray(bq, np.float32),
        np.asarray(Wk, np.float32),
        np.asarray(bk, np.float32),
        np.asarray(Wv, np.float32),
        np.asarray(bv, np.float32),
        np.asarray(Wo, np.float32),
        np.asarray(bo, np.float32),
    )
    res = bass_utils.run_bass_kernel_spmd(nc, in_maps, core_ids=list(range(8)))
    parts = [res.results[c]["out"] for c in range(8)]
    bo = np.asarray(bo, np.float32)
    out = np.stack(
        [
            parts[0] + parts[1] + parts[2] + parts[3] + bo,
            parts[4] + parts[5] + parts[6] + parts[7] + bo,
        ]
    )
    return out.astype(np.float32)


# revision 12
# speedup vs baseline: 1.4586x; 1.4586x over previous
"""Multi-head attention (B=2, S=2048, D=1024, H=16) on 8 Trainium2 cores.

Sharding: core c handles batch b=c//4, head-group g=c%4 (4 heads = 256 QKV
columns).  Each core computes its head-group's QKV projections, attention,
and a partial output projection (its 256 rows of Wo); the host sums the 4
partials per batch and adds bo.

Device pipeline (per core):
  xqT/xkT/xvT [1024, 2048] f16  inputs pre-transposed + cast on host
  Qt, Kt  [dcol, s] f32r SBUF [128, 2, 2048]  (head h at partitions
          (h%2)*64..+64, free-tile h//2; f32 PSUM rounded on eviction)
  V       [s, dcol] f16 SBUF [128, 16, 4, 65]; col 64 = 1.0 (the ones
          column makes att@V also emit the softmax denominator)
  S^T = Kt'Q  PSUM [k, q] -> exp on ScalarE (scale=1/8) -> P^T f16
  O = P'V     PSUM [q, 65] -> normalize by col 64 on VectorE -> A [s, 256]
  A^T via PE transpose; out = A^T' Wo (f32r) -> partial [2048, 1024]

Ordering is tuned for the in-order engine streams: Q/K projections first
(so exp starts as soon as their DMA lands), V projection emitted between
the first three heads' score/exp blocks and the first att@V, output tail
for q-chunk qc emitted inside qc+1.
"""

import sys

sys.path.insert(0, "/opt/trn_rl_repo")

import numpy as np

import concourse.bacc as bacc
import concourse.mybir as mybir
from concourse import bass_utils, tile
from concourse.masks import make_identity

P = 128
S = 2048  # sequence length
DM = 1024  # d_model
GC = 256  # QKV columns per core (4 heads x 64)
NHC = 4  # heads per core
DEP = 64  # head depth
NKT = DM // P  # 8 contraction tiles over d_model
NST = S // P  # 16 s-tiles of 128
NSC = S // 512  # 4 s-chunks of 512
f32 = mybir.dt.float32
f32r = mybir.dt.float32r
f16 = mybir.dt.float16
EXP = mybir.ActivationFunctionType.Exp

_nc_cache = {}

QK_DT = f16  # dtype of Qt/Kt score operands
AT_DT = f16  # dtype of A^T / Wo outproj operands


def build_nc(loop_iters=None):
    key = (loop_iters, QK_DT, AT_DT)
    if key in _nc_cache:
        return _nc_cache[key]
    from contextlib import ExitStack

    nc = bacc.Bacc("TRN2", target_bir_lowering=False, debug=False)
    xqT = nc.dram_tensor("xqT", [DM, S], f16, kind="ExternalInput").ap()
    xkT = nc.dram_tensor("xkT", [DM, S], f16, kind="ExternalInput").ap()
    xvT = nc.dram_tensor("xvT", [DM, S], f16, kind="ExternalInput").ap()
    wq = nc.dram_tensor("wq", [DM, GC], f16, kind="ExternalInput").ap()
    wk = nc.dram_tensor("wk", [DM, GC], f16, kind="ExternalInput").ap()
    wv = nc.dram_tensor("wv", [DM, GC], f16, kind="ExternalInput").ap()
    wo = nc.dram_tensor("wo", [GC, DM], AT_DT, kind="ExternalInput").ap()
    bq = nc.dram_tensor("bq", [GC], f32, kind="ExternalInput").ap()
    bk = nc.dram_tensor("bk", [GC], f32, kind="ExternalInput").ap()
    bv = nc.dram_tensor("bv", [GC], f32, kind="ExternalInput").ap()
    out = nc.dram_tensor("out", [S, DM], f32, kind="ExternalOutput").ap()

    with tile.TileContext(nc) as tc, ExitStack() as ctx:
        consts = ctx.enter_context(tc.tile_pool(name="consts", bufs=1))
        wq_sb = consts.tile([P, NKT, GC], f16, tag="wq")
        wk_sb = consts.tile([P, NKT, GC], f16, tag="wk")
        wv_sb = consts.tile([P, NKT, GC], f16, tag="wv")
        wo_sb = consts.tile([P, 2, DM], AT_DT, tag="wo")
        # Biases as K=128 matmul operands: bias data in partition-row 0,
        # zeros elsewhere, so accumulation stays in 128x128 array mode.
        bq_sb = consts.tile([P, GC], f16, tag="bq")
        bk_sb = consts.tile([P, GC], f16, tag="bk")
        bv_sb = consts.tile([P, GC], f16, tag="bv")
        stage = consts.tile([P, GC], f32, tag="stage")

        def load_weight(w_sb, w, eng=None):
            (eng or nc.sync).dma_start(w_sb[:], w.rearrange("(kt p) m -> p kt m", p=P))

        def load_bias(b_sb, b, eng=None):
            nc.vector.memset(stage[:], 0.0)
            (eng or nc.sync).dma_start(stage[0:1, :], b[None, :])
            nc.vector.tensor_copy(b_sb[:], stage[:])

        ones_big = consts.tile([P, 512], f16, tag="ones")
        nc.vector.memset(ones_big[:], 1.0)
        e0row = consts.tile([P, P], f16, tag="e0")
        nc.vector.memset(e0row[:], 0.0)
        nc.vector.memset(e0row[0:1, :], 1.0)
        ident = consts.tile([P, P], f32, tag="ident")
        make_identity(nc, ident)

        res = ctx.enter_context(tc.tile_pool(name="res", bufs=1))
        qt_ch = [res.tile([P, 2, 512], QK_DT, tag=f"qt{i}", name=f"qtc{i}") for i in range(NSC)]
        kt_ch = [res.tile([P, 2, 512], QK_DT, tag=f"kt{i}", name=f"ktc{i}") for i in range(NSC)]
        v_sb = res.tile([P, NST, NHC, DEP + 1], f16, tag="v")
        a_sb = res.tile([P, NST, GC], f32, tag="a")
        at_sb = res.tile([P, 2, S], AT_DT, tag="at")
        nc.vector.memset(v_sb[:, :, :, DEP], 1.0)

        xs_pool = ctx.enter_context(tc.tile_pool(name="xs", bufs=2))
        xr_pool = ctx.enter_context(tc.tile_pool(name="xr", bufs=8))
        pt_pool = ctx.enter_context(tc.tile_pool(name="pt", bufs=2))
        rc_pool = ctx.enter_context(tc.tile_pool(name="rc", bufs=8))
        ob_pool = ctx.enter_context(tc.tile_pool(name="ob", bufs=2))

        def body():
            # ---- Q/K projections.  Q: kt-outer, xq streamed on the SP
            # queue, all 8 PSUM banks.  K: ns-outer over resident xk tiles
            # (Pool queue) so each Kt chunk evicts as soon as its two psum
            # slots free up and the exp stream can start ~20us in. ----
            with tc.tile_pool(name="pj_psum", bufs=8, space="PSUM") as pj_psum:
                load_weight(wq_sb, wq, nc.sync)
                load_bias(bq_sb, bq, nc.sync)
                load_weight(wk_sb, wk, nc.gpsimd)
                load_bias(bk_sb, bk, nc.gpsimd)
                xks = []
                for kt in range(NKT):
                    xt = xr_pool.tile([P, S], f16, tag="xr", name=f"xk{kt}")
                    nc.gpsimd.dma_start(xt[:], xkT[kt * P : (kt + 1) * P, :])
                    xks.append(xt)
                ps = [
                    pj_psum.tile([P, 512], f32, tag="pj", name=f"pjq{i}")
                    for i in range(8)
                ]
                for kt in range(NKT):
                    xt = xs_pool.tile([P, S], f16, tag="x", name=f"xq{kt}")
                    nc.sync.dma_start(xt[:], xqT[kt * P : (kt + 1) * P, :])
                    for ns in range(NSC):
                        for mt in range(2):
                            nc.tensor.matmul(
                                ps[ns * 2 + mt][:],
                                lhsT=wq_sb[:, kt, mt * P : (mt + 1) * P],
                                rhs=xt[:, ns * 512 : (ns + 1) * 512],
                                start=(kt == 0),
                                stop=False,
                            )
                for ns in range(NSC):
                    for mt in range(2):
                        nc.tensor.matmul(
                            ps[ns * 2 + mt][:],
                            lhsT=bq_sb[:, mt * P : (mt + 1) * P],
                            rhs=ones_big[:],
                            start=False,
                            stop=True,
                        )
                        nc.vector.tensor_copy(
                            qt_ch[ns][:, mt, :], ps[ns * 2 + mt][:]
                        )
                for ns in range(NSC):
                    for mt in range(2):
                        pk = pj_psum.tile([P, 512], f32, tag="pj", name=f"pk{ns}{mt}")
                        for kt in range(NKT):
                            nc.tensor.matmul(
                                pk[:],
                                lhsT=wk_sb[:, kt, mt * P : (mt + 1) * P],
                                rhs=xks[kt][:, ns * 512 : (ns + 1) * 512],
                                start=(kt == 0),
                                stop=False,
                            )
                        nc.tensor.matmul(
                            pk[:],
                            lhsT=bk_sb[:, mt * P : (mt + 1) * P],
                            rhs=ones_big[:],
                            start=False,
                            stop=True,
                        )
                        nc.vector.tensor_copy(kt_ch[ns][:, mt, :], pk[:])

            # ---- attention (qc-outer) with V projection and output tail
            #      interleaved ----
            with (
                tc.tile_pool(name="st_psum", bufs=2, space="PSUM") as st_psum,
                tc.tile_pool(name="o_psum", bufs=2, space="PSUM") as o_psum,
                tc.tile_pool(name="t_psum", bufs=2, space="PSUM") as t_psum,
            ):
                ptts = {}

                def emit_st_pair(qc, mt):
                    # heads 2*mt (partitions 0:64, PE tile T0) and 2*mt+1
                    # (partitions 64:128, tile T8): consecutive matmuls land
                    # on disjoint row-strips, so the PE array runs them
                    # concurrently in 64x128 tiling mode.  One exp per
                    # k-tile covers both heads' [128, 512] score tiles.
                    ptt = pt_pool.tile(
                        [P, NST, 2, 512], f16, tag="pt", name=f"pt{qc}{mt}"
                    )
                    ptts[(qc, mt)] = ptt
                    for kt in range(NST):
                        stt = st_psum.tile(
                            [P, 2, 512], f32, tag="st", name=f"st{qc}{mt}{kt}"
                        )
                        for hp in range(2):
                            pb = hp * DEP
                            nc.tensor.matmul(
                                stt[:, hp, :],
                                lhsT=kt_ch[kt // 4][pb : pb + DEP, mt, (kt % 4) * P : (kt % 4 + 1) * P],
                                rhs=qt_ch[qc][pb : pb + DEP, mt, :],
                                start=True,
                                stop=True,
                            )
                        nc.scalar.activation(
                            ptt[:, kt, :, :], stt[:], EXP, scale=0.125
                        )

                def emit_attv(qc, h):
                    ptt = ptts[(qc, h // 2)]
                    if h % 2 == 1:
                        ptts.pop((qc, h // 2))
                    ot = o_psum.tile([P, 4, DEP + 1], f32, tag="o", name=f"o{qc}{h}")
                    for qt in range(4):
                        for kt in range(NST):
                            nc.tensor.matmul(
                                ot[:, qt, :],
                                lhsT=ptt[:, kt, h % 2, qt * P : (qt + 1) * P],
                                rhs=v_sb[:, kt, h, :],
                                start=(kt == 0),
                                stop=(kt == NST - 1),
                            )
                    for qt in range(4):
                        st_i = qc * 4 + qt
                        rc = rc_pool.tile([P, 1], f32, tag="rc", name=f"rc{qc}{h}{qt}")
                        nc.vector.reciprocal(rc[:], ot[:, qt, DEP : DEP + 1])
                        nc.vector.tensor_scalar_mul(
                            a_sb[:, st_i, h * DEP : (h + 1) * DEP],
                            ot[:, qt, 0:DEP],
                            rc[:],
                        )

                def emit_vproj():
                    load_weight(wv_sb, wv, nc.gpsimd)
                    load_bias(bv_sb, bv, nc.gpsimd)
                    xvs = []
                    for kt in range(NKT):
                        xt = xr_pool.tile([P, S], f16, tag="xr", name=f"xv{kt}")
                        nc.gpsimd.dma_start(xt[:], xvT[kt * P : (kt + 1) * P, :])
                        xvs.append(xt)
                    for st in range(NST):
                        vp = o_psum.tile([P, GC], f32, tag="o", name=f"vp{st}")
                        for kt in range(NKT):
                            nc.tensor.matmul(
                                vp[:],
                                lhsT=xvs[kt][:, st * P : (st + 1) * P],
                                rhs=wv_sb[:, kt, :],
                                start=(kt == 0),
                                stop=False,
                            )
                        nc.tensor.matmul(
                            vp[:], lhsT=e0row[:], rhs=bv_sb[:], start=False, stop=True
                        )
                        nc.vector.tensor_copy(
                            v_sb[:, st, :, 0:DEP],
                            vp[:].rearrange("p (h d) -> p h d", h=NHC),
                        )

                wo_loaded = [False]

                def emit_tail(qc):
                    if not wo_loaded[0]:
                        nc.sync.dma_start(
                            wo_sb[:], wo.rearrange("(mt p) n -> p mt n", p=P)
                        )
                        wo_loaded[0] = True
                    for j in range(4):
                        st = qc * 4 + j
                        for mt in range(2):
                            tp = t_psum.tile([P, P], f32, tag="t", name=f"tp{st}{mt}")
                            nc.tensor.transpose(
                                tp[:], a_sb[:, st, mt * P : (mt + 1) * P], ident[:]
                            )
                            nc.vector.tensor_copy(
                                at_sb[:, mt, st * P : (st + 1) * P], tp[:]
                            )
                        ob = ob_pool.tile([P, DM], f32, tag="ob", name=f"ob{st}")
                        for oh in range(2):
                            op = t_psum.tile([P, 512], f32, tag="t", name=f"op{st}{oh}")
                            for mt in range(2):
                                nc.tensor.matmul(
                                    op[:],
                                    lhsT=at_sb[:, mt, st * P : (st + 1) * P],
                                    rhs=wo_sb[:, mt, oh * 512 : (oh + 1) * 512],
                                    start=(mt == 0),
                                    stop=(mt == 1),
                                )
                            nc.vector.tensor_copy(
                                ob[:, oh * 512 : (oh + 1) * 512], op[:]
                            )
                        nc.sync.dma_start(out[st * P : (st + 1) * P, :], ob[:])

                # Software-pipelined emission: keep the exp stream dense by
                # emitting each head-pair's scores one pair ahead of its
                # att@V (in-order PE stream), V projection before the first
                # att@V, and each q-chunk's output tail after its last head.
                prs = [(qc, mt) for qc in range(NSC) for mt in range(2)]
                emit_st_pair(*prs[0])
                emit_st_pair(*prs[1])
                emit_vproj()
                si = 2
                for qc, mt in prs:
                    emit_attv(qc, 2 * mt)
                    emit_attv(qc, 2 * mt + 1)
                    if mt == 1:
                        emit_tail(qc)
                    if si < len(prs):
                        emit_st_pair(*prs[si])
                        si += 1

        if loop_iters is None:
            body()
        else:
            with tc.For_i(0, loop_iters, 1):
                body()

    nc.compile()
    _nc_cache[key] = nc
    return nc


def make_in_maps(query, key, value, Wq, bq, Wk, bk, Wv, bv, Wo, bo):
    qT = [np.ascontiguousarray(query[b].T.astype(np.float16)) for b in range(2)]
    kT = [np.ascontiguousarray(key[b].T.astype(np.float16)) for b in range(2)]
    vT = [np.ascontiguousarray(value[b].T.astype(np.float16)) for b in range(2)]
    Wq16, Wk16, Wv16 = (np.asarray(W, np.float16) for W in (Wq, Wk, Wv))
    in_maps = []
    for c in range(8):
        b, g = divmod(c, 4)
        sl = slice(g * GC, (g + 1) * GC)
        in_maps.append(
            {
                "xqT": qT[b],
                "xkT": kT[b],
                "xvT": vT[b],
                "wq": np.ascontiguousarray(Wq16[:, sl]),
                "wk": np.ascontiguousarray(Wk16[:, sl]),
                "wv": np.ascontiguousarray(Wv16[:, sl]),
                "wo": np.ascontiguousarray(np.asarray(Wo, mybir.dt.np(AT_DT))[sl, :]),
                "bq": np.ascontiguousarray(np.asarray(bq, np.float32)[sl]),
                "bk": np.ascontiguousarray(np.asarray(bk, np.float32)[sl]),
                "bv": np.ascontiguousarray(np.asarray(bv, np.float32)[sl]),
            }
        )
    return in_maps


def kernel(query, key, value, Wq, bq, Wk, bk, Wv, bv, Wo, bo):
    nc = build_nc()
    in_maps = make_in_maps(query, key, value, Wq, bq, Wk, bk, Wv, bv, Wo, bo)
    res = bass_utils.run_bass_kernel_spmd(nc, in_maps, core_ids=list(range(8)))
    parts = [res.results[c]["out"] for c in range(8)]
    bo = np.asarray(bo, np.float32)
    out = np.stack(
        [
            parts[0] + parts[1] + parts[2] + parts[3] + bo,
            parts[4] + parts[5] + parts[6] + parts[7] + bo,
        ]
    )
    return out.astype(np.float32)


# revision 14
# speedup vs baseline: 1.6500x; 1.1312x over previous
"""Multi-head attention (B=2, S=2048, D=1024, H=16) on 8 Trainium2 cores.

Sharding: core c handles batch b=c//4, head-group g=c%4 (4 heads = 256 QKV
columns).  Each core computes its head-group's QKV projections, attention,
and a partial output projection (its 256 rows of Wo); the host sums the 4
partials per batch and adds bo.

Device pipeline (per core):
  xqT/xkT/xvT [1024, 2048] f16  inputs pre-transposed + cast on host
  Qt, Kt  [dcol, s] f32r SBUF [128, 2, 2048]  (head h at partitions
          (h%2)*64..+64, free-tile h//2; f32 PSUM rounded on eviction)
  V       [s, dcol] f16 SBUF [128, 16, 4, 65]; col 64 = 1.0 (the ones
          column makes att@V also emit the softmax denominator)
  S^T = Kt'Q  PSUM [k, q] -> exp on ScalarE (scale=1/8) -> P^T f16
  O = P'V     PSUM [q, 65] -> normalize by col 64 on VectorE -> A [s, 256]
  A^T via PE transpose; out = A^T' Wo (f32r) -> partial [2048, 1024]

Ordering is tuned for the in-order engine streams: Q/K projections first
(so exp starts as soon as their DMA lands), V projection emitted between
the first three heads' score/exp blocks and the first att@V, output tail
for q-chunk qc emitted inside qc+1.
"""

import sys

sys.path.insert(0, "/opt/trn_rl_repo")

import numpy as np

import concourse.bacc as bacc
import concourse.mybir as mybir
from concourse import bass_utils, tile
from concourse.masks import make_identity

P = 128
S = 2048  # sequence length
DM = 1024  # d_model
GC = 256  # QKV columns per core (4 heads x 64)
NHC = 4  # heads per core
DEP = 64  # head depth
NKT = DM // P  # 8 contraction tiles over d_model
NST = S // P  # 16 s-tiles of 128
NSC = S // 512  # 4 s-chunks of 512
f32 = mybir.dt.float32
f32r = mybir.dt.float32r
f16 = mybir.dt.float16
EXP = mybir.ActivationFunctionType.Exp

_nc_cache = {}

QK_DT = f16  # dtype of Qt/Kt score operands
AT_DT = f16  # dtype of A^T / Wo outproj operands


def build_nc(loop_iters=None):
    key = (loop_iters, QK_DT, AT_DT)
    if key in _nc_cache:
        return _nc_cache[key]
    from contextlib import ExitStack

    nc = bacc.Bacc("TRN2", target_bir_lowering=False, debug=False)
    xqT = nc.dram_tensor("xqT", [DM, S], f16, kind="ExternalInput").ap()
    xkT = nc.dram_tensor("xkT", [DM, S], f16, kind="ExternalInput").ap()
    xvT = nc.dram_tensor("xvT", [DM, S], f16, kind="ExternalInput").ap()
    wq = nc.dram_tensor("wq", [DM, GC], f16, kind="ExternalInput").ap()
    wk = nc.dram_tensor("wk", [DM, GC], f16, kind="ExternalInput").ap()
    wv = nc.dram_tensor("wv", [DM, GC], f16, kind="ExternalInput").ap()
    wo = nc.dram_tensor("wo", [GC, DM], AT_DT, kind="ExternalInput").ap()
    bq = nc.dram_tensor("bq", [GC], f32, kind="ExternalInput").ap()
    bk = nc.dram_tensor("bk", [GC], f32, kind="ExternalInput").ap()
    bv = nc.dram_tensor("bv", [GC], f32, kind="ExternalInput").ap()
    out = nc.dram_tensor("out", [S, DM], f32, kind="ExternalOutput").ap()

    with tile.TileContext(nc) as tc, ExitStack() as ctx:
        consts = ctx.enter_context(tc.tile_pool(name="consts", bufs=1))
        wq_sb = consts.tile([P, NKT, GC], f16, tag="wq")
        wk_sb = consts.tile([P, NKT, GC], f16, tag="wk")
        wv_sb = consts.tile([P, NKT, GC], f16, tag="wv")
        wo_sb = consts.tile([P, 2, DM], AT_DT, tag="wo")
        # Biases as K=128 matmul operands: bias data in partition-row 0,
        # zeros elsewhere, so accumulation stays in 128x128 array mode.
        bq_sb = consts.tile([P, GC], f16, tag="bq")
        bk_sb = consts.tile([P, GC], f16, tag="bk")
        bv_sb = consts.tile([P, GC], f16, tag="bv")
        stage = consts.tile([P, GC], f32, tag="stage")

        def load_weight(w_sb, w, eng=None):
            (eng or nc.sync).dma_start(w_sb[:], w.rearrange("(kt p) m -> p kt m", p=P))

        def load_bias(b_sb, b, eng=None):
            nc.vector.memset(stage[:], 0.0)
            (eng or nc.sync).dma_start(stage[0:1, :], b[None, :])
            nc.vector.tensor_copy(b_sb[:], stage[:])

        ones_big = consts.tile([P, 512], f16, tag="ones")
        nc.vector.memset(ones_big[:], 1.0)
        e0row = consts.tile([P, P], f16, tag="e0")
        nc.vector.memset(e0row[:], 0.0)
        nc.vector.memset(e0row[0:1, :], 1.0)
        ident = consts.tile([P, P], f32, tag="ident")
        make_identity(nc, ident)

        res = ctx.enter_context(tc.tile_pool(name="res", bufs=1))
        qt_ch = [res.tile([P, 2, 512], QK_DT, tag=f"qt{i}", name=f"qtc{i}") for i in range(NSC)]
        kt_ch = [res.tile([P, 2, 512], QK_DT, tag=f"kt{i}", name=f"ktc{i}") for i in range(NSC)]
        v_sb = res.tile([P, NST, NHC, DEP + 1], f16, tag="v")
        a_sb = res.tile([P, NST, GC], f32, tag="a")
        at_sb = res.tile([P, 2, S], AT_DT, tag="at")
        nc.vector.memset(v_sb[:, :, :, DEP], 1.0)

        xs_pool = ctx.enter_context(tc.tile_pool(name="xs", bufs=2))
        xr_pool = ctx.enter_context(tc.tile_pool(name="xr", bufs=8))
        xv4_pool = ctx.enter_context(tc.tile_pool(name="xv4", bufs=4))
        pt_pool = ctx.enter_context(tc.tile_pool(name="pt", bufs=2))
        rc_pool = ctx.enter_context(tc.tile_pool(name="rc", bufs=8))
        ob_pool = ctx.enter_context(tc.tile_pool(name="ob", bufs=2))

        def body():
            # ---- Q/K projections.  Q: kt-outer, xq streamed on the SP
            # queue, all 8 PSUM banks.  K: ns-outer over resident xk tiles
            # (Pool queue) so each Kt chunk evicts as soon as its two psum
            # slots free up and the exp stream can start ~20us in. ----
            xks = []
            with tc.tile_pool(name="pj_psum", bufs=8, space="PSUM") as pj_psum:
                load_weight(wq_sb, wq, nc.sync)
                load_bias(bq_sb, bq, nc.sync)
                load_weight(wk_sb, wk, nc.gpsimd)
                load_bias(bk_sb, bk, nc.gpsimd)
                for kt in range(NKT):
                    xt = xr_pool.tile([P, S], f16, tag="xr", name=f"xk{kt}")
                    nc.gpsimd.dma_start(xt[:], xkT[kt * P : (kt + 1) * P, :])
                    xks.append(xt)
                ps = [
                    pj_psum.tile([P, 512], f32, tag="pj", name=f"pjq{i}")
                    for i in range(8)
                ]
                for kt in range(NKT):
                    xt = xs_pool.tile([P, S], f16, tag="x", name=f"xq{kt}")
                    nc.sync.dma_start(xt[:], xqT[kt * P : (kt + 1) * P, :])
                    for ns in range(NSC):
                        for mt in range(2):
                            nc.tensor.matmul(
                                ps[ns * 2 + mt][:],
                                lhsT=wq_sb[:, kt, mt * P : (mt + 1) * P],
                                rhs=xt[:, ns * 512 : (ns + 1) * 512],
                                start=(kt == 0),
                                stop=False,
                            )
                for ns in range(NSC):
                    for mt in range(2):
                        nc.tensor.matmul(
                            ps[ns * 2 + mt][:],
                            lhsT=bq_sb[:, mt * P : (mt + 1) * P],
                            rhs=ones_big[:],
                            start=False,
                            stop=True,
                        )
                        nc.vector.tensor_copy(
                            qt_ch[ns][:, mt, :], ps[ns * 2 + mt][:]
                        )
            # ---- attention (qc-outer) with K projection, V projection and
            #      output tail interleaved.  kt_psum opens first so it lands
            #      on the banks of Q's first-evicted psum slots. ----
            with (
                tc.tile_pool(name="kt_psum", bufs=2, space="PSUM") as kt_psum,
                tc.tile_pool(name="st_psum", bufs=2, space="PSUM") as st_psum,
                tc.tile_pool(name="o_psum", bufs=2, space="PSUM") as o_psum,
            ):
                def emit_kproj_ns(ns):
                    for mt in range(2):
                        pk = kt_psum.tile([P, 512], f32, tag="kt", name=f"pk{ns}{mt}")
                        for kt in range(NKT):
                            nc.tensor.matmul(
                                pk[:],
                                lhsT=wk_sb[:, kt, mt * P : (mt + 1) * P],
                                rhs=xks[kt][:, ns * 512 : (ns + 1) * 512],
                                start=(kt == 0),
                                stop=False,
                            )
                        nc.tensor.matmul(
                            pk[:],
                            lhsT=bk_sb[:, mt * P : (mt + 1) * P],
                            rhs=ones_big[:],
                            start=False,
                            stop=True,
                        )
                        nc.vector.tensor_copy(kt_ch[ns][:, mt, :], pk[:])
                ptts = {}

                def emit_st_pair(qc, mt, kts=None):
                    # heads 2*mt (partitions 0:64, PE tile T0) and 2*mt+1
                    # (partitions 64:128, tile T8): consecutive matmuls land
                    # on disjoint row-strips, so the PE array runs them
                    # concurrently in 64x128 tiling mode.  One exp per
                    # k-tile covers both heads' [128, 512] score tiles.
                    if (qc, mt) in ptts:
                        ptt = ptts[(qc, mt)]
                    else:
                        ptt = pt_pool.tile(
                            [P, NST, 2, 512], f16, tag="pt", name=f"pt{qc}{mt}"
                        )
                        ptts[(qc, mt)] = ptt
                    for kt in kts if kts is not None else range(NST):
                        stt = st_psum.tile(
                            [P, 2, 512], f32, tag="st", name=f"st{qc}{mt}{kt}"
                        )
                        for hp in range(2):
                            pb = hp * DEP
                            nc.tensor.matmul(
                                stt[:, hp, :],
                                lhsT=kt_ch[kt // 4][pb : pb + DEP, mt, (kt % 4) * P : (kt % 4 + 1) * P],
                                rhs=qt_ch[qc][pb : pb + DEP, mt, :],
                                start=True,
                                stop=True,
                            )
                        nc.scalar.activation(
                            ptt[:, kt, :, :], stt[:], EXP, scale=0.125
                        )

                def emit_attv(qc, h):
                    ptt = ptts[(qc, h // 2)]
                    if h % 2 == 1:
                        ptts.pop((qc, h // 2))
                    ot = o_psum.tile([P, 4, DEP + 1], f32, tag="o", name=f"o{qc}{h}")
                    for qt in range(4):
                        for kt in range(NST):
                            nc.tensor.matmul(
                                ot[:, qt, :],
                                lhsT=ptt[:, kt, h % 2, qt * P : (qt + 1) * P],
                                rhs=v_sb[:, kt, h, :],
                                start=(kt == 0),
                                stop=(kt == NST - 1),
                            )
                    for qt in range(4):
                        st_i = qc * 4 + qt
                        rc = rc_pool.tile([P, 1], f32, tag="rc", name=f"rc{qc}{h}{qt}")
                        nc.vector.reciprocal(rc[:], ot[:, qt, DEP : DEP + 1])
                        nc.vector.tensor_scalar_mul(
                            a_sb[:, st_i, h * DEP : (h + 1) * DEP],
                            ot[:, qt, 0:DEP],
                            rc[:],
                        )

                def emit_vproj():
                    load_weight(wv_sb, wv, nc.gpsimd)
                    load_bias(bv_sb, bv, nc.gpsimd)
                    xvs = []
                    for kt in range(NKT):
                        pool, tg = (xv4_pool, "xv4") if kt < 4 else (xr_pool, "xr")
                        xt = pool.tile([P, S], f16, tag=tg, name=f"xv{kt}")
                        nc.gpsimd.dma_start(xt[:], xvT[kt * P : (kt + 1) * P, :])
                        xvs.append(xt)
                    for st in range(NST):
                        vp = o_psum.tile([P, GC], f32, tag="o", name=f"vp{st}")
                        for kt in range(NKT):
                            nc.tensor.matmul(
                                vp[:],
                                lhsT=xvs[kt][:, st * P : (st + 1) * P],
                                rhs=wv_sb[:, kt, :],
                                start=(kt == 0),
                                stop=False,
                            )
                        nc.tensor.matmul(
                            vp[:], lhsT=e0row[:], rhs=bv_sb[:], start=False, stop=True
                        )
                        nc.vector.tensor_copy(
                            v_sb[:, st, :, 0:DEP],
                            vp[:].rearrange("p (h d) -> p h d", h=NHC),
                        )

                wo_loaded = [False]

                def emit_tail(qc):
                    if not wo_loaded[0]:
                        nc.sync.dma_start(
                            wo_sb[:], wo.rearrange("(mt p) n -> p mt n", p=P)
                        )
                        wo_loaded[0] = True
                    for j in range(4):
                        st = qc * 4 + j
                        for mt in range(2):
                            tp = kt_psum.tile([P, P], f32, tag="kt", name=f"tp{st}{mt}")
                            nc.tensor.transpose(
                                tp[:], a_sb[:, st, mt * P : (mt + 1) * P], ident[:]
                            )
                            nc.vector.tensor_copy(
                                at_sb[:, mt, st * P : (st + 1) * P], tp[:]
                            )
                        ob = ob_pool.tile([P, DM], f32, tag="ob", name=f"ob{st}")
                        for oh in range(2):
                            op = kt_psum.tile([P, 512], f32, tag="kt", name=f"op{st}{oh}")
                            for mt in range(2):
                                nc.tensor.matmul(
                                    op[:],
                                    lhsT=at_sb[:, mt, st * P : (st + 1) * P],
                                    rhs=wo_sb[:, mt, oh * 512 : (oh + 1) * 512],
                                    start=(mt == 0),
                                    stop=(mt == 1),
                                )
                            nc.vector.tensor_copy(
                                ob[:, oh * 512 : (oh + 1) * 512], op[:]
                            )
                        nc.sync.dma_start(out[st * P : (st + 1) * P, :], ob[:])

                # Software-pipelined emission for the in-order engine
                # streams: K projection ns-blocks woven with the first
                # head-pair's score matmuls (exp starts ~10us earlier), V
                # projection before the first att@V, each pair's scores one
                # pair ahead of its att@V, output tails after each q-chunk.
                emit_kproj_ns(0)
                emit_kproj_ns(1)
                emit_st_pair(0, 0, range(0, 4))
                emit_kproj_ns(2)
                emit_st_pair(0, 0, range(4, 8))
                emit_kproj_ns(3)
                emit_st_pair(0, 0, range(8, 16))
                emit_st_pair(0, 1)
                emit_vproj()
                prs = [(qc, mt) for qc in range(NSC) for mt in range(2)]
                si = 2
                for qc, mt in prs:
                    emit_attv(qc, 2 * mt)
                    if si < len(prs):
                        emit_st_pair(*prs[si])
                        si += 1
                    emit_attv(qc, 2 * mt + 1)
                    if mt == 1:
                        emit_tail(qc)

        if loop_iters is None:
            body()
        else:
            with tc.For_i(0, loop_iters, 1):
                body()

    nc.compile()
    _nc_cache[key] = nc
    return nc


def make_in_maps(query, key, value, Wq, bq, Wk, bk, Wv, bv, Wo, bo):
    qT = [np.ascontiguousarray(query[b].T.astype(np.float16)) for b in range(2)]
    kT = [np.ascontiguousarray(key[b].T.astype(np.float16)) for b in range(2)]
    vT = [np.ascontiguousarray(value[b].T.astype(np.float16)) for b in range(2)]
    Wq16, Wk16, Wv16 = (np.asarray(W, np.float16) for W in (Wq, Wk, Wv))
    in_maps = []
    for c in range(8):
        b, g = divmod(c, 4)
        sl = slice(g * GC, (g + 1) * GC)
        in_maps.append(
            {
                "xqT": qT[b],
                "xkT": kT[b],
                "xvT": vT[b],
                "wq": np.ascontiguousarray(Wq16[:, sl]),
                "wk": np.ascontiguousarray(Wk16[:, sl]),
                "wv": np.ascontiguousarray(Wv16[:, sl]),
                "wo": np.ascontiguousarray(np.asarray(Wo, mybir.dt.np(AT_DT))[sl, :]),
                "bq": np.ascontiguousarray(np.asarray(bq, np.float32)[sl]),
                "bk": np.ascontiguousarray(np.asarray(bk, np.float32)[sl]),
                "bv": np.ascontiguousarray(np.asarray(bv, np.float32)[sl]),
            }
        )
    return in_maps


def kernel(query, key, value, Wq, bq, Wk, bk, Wv, bv, Wo, bo):
    nc = build_nc()
    in_maps = make_in_maps(query, key, value, Wq, bq, Wk, bk, Wv, bv, Wo, bo)
    res = bass_utils.run_bass_kernel_spmd(nc, in_maps, core_ids=list(range(8)))
    parts = [res.results[c]["out"] for c in range(8)]
    bo = np.asarray(bo, np.float32)
    out = np.stack(
        [
            parts[0] + parts[1] + parts[2] + parts[3] + bo,
            parts[4] + parts[5] + parts[6] + parts[7] + bo,
        ]
    )
    return out.astype(np.float32)
